# revision 4
# baseline (speedup 1.0000x reference)
"""Trainium2 Bass kernel for nn_BiRNN (2-layer bidirectional tanh RNN classifier).

Strategy
--------
The output depends only on the final hidden state of the top layer in each
direction, but the tanh recurrence is strictly sequential in time.  We
restructure the per-direction compute as:

  P0: zx0[t] = emb_x[t] @ W0_ih + (b0_ih + b0_hh)     -- big parallel GEMM
  S1: h0[t]  = tanh(zx0[t] + h0[t-1] @ W0_hh)          -- serial, 512 steps
  P1: zh1[t] = h0[t] @ W1_ih + (b1_ih + b1_hh)         -- big parallel GEMM
  S2: h1[t]  = tanh(zh1[t] + h1[t-1] @ W1_hh)          -- serial, 512 steps

Only the h @ W_hh recurrences stay on the serial critical path.  Everything is
kept in *transposed* layout (hT: [H, B] with H on partitions) so that each
serial step is: stationary = W_hh 128x128 chunks (fp16, fast weight load),
moving = hT chunks, output = next hT directly -- no per-step transposes, and
biases become per-partition scalars folded into the precomputed zx.

The zx[t] term is preloaded into PSUM with an identity-stationary matmul
(start=True), so the 64 accumulating weight matmuls then add onto it; tanh is
applied by ScalarE straight out of PSUM (fp32 internal, 4-ULP table).

Parallelization: per-step collectives are far too slow on this hardware
(multi-us floor), so the two directions run on disjoint cores and the batch is
split 4-ways to shrink the per-core parallel-GEMM phases:
  cores 0-3: forward direction,  batch rows 16c   .. 16c+15
  cores 4-7: backward direction, batch rows 16(c-4) .. 16(c-4)+15
Each core runs the full P0/S1/P1/S2 chain on its shard; no cross-core
communication.  The tiny FC head (8.4 MFLOP) is applied on the host during
unsharding.

Numerics: fp16 operand storage with fp32 PSUM accumulation measures ~1.3e-4
max relative error on the final [64, 2] output vs the fp32 reference.
"""

import os
import sys

import numpy as np

for _p in ("/opt/trn_rl_repo",):
    if _p not in sys.path:
        sys.path.insert(0, _p)

import concourse.bass as bass
import concourse.mybir as mybir
import concourse.tile as tile
from concourse import bacc
from concourse.bass_utils import run_bass_kernel_spmd

# Problem constants (hardcoded per the spec).
B, S, V, E, H, C = 64, 512, 32000, 512, 1024, 2
NCORES = 8
BL = B // 4          # batch rows per core (4-way batch split per direction)
EC = E // 128        # 4   K-chunks for the E-contraction
KC = H // 128        # 8   K-chunks for the H-contraction
MC = H // 128        # 8   output (H) chunks
SBLK = 16            # serial-phase zx streaming block (steps)
TBLK = 32            # parallel-phase block (steps) -> 512 moving columns
F16 = mybir.dt.float16
F32 = mybir.dt.float32
TANH = mybir.ActivationFunctionType.Tanh

_programs: dict = {}   # nsteps -> Bass program
last_results = None    # BassKernelResults of the most recent run (for test.py)


def _pphase(tc, nc, ctx, name, w_sb, kc, mov_dram, mov_is_emb, out_dram, bias_sb,
            nsteps):
    """out[t] = mov[t] @ W + bias, all in transposed/chunked layout."""
    movp = ctx.enter_context(tc.tile_pool(name=f"{name}_mov", bufs=3))
    stp = ctx.enter_context(tc.tile_pool(name=f"{name}_stg", bufs=2))
    psp = ctx.enter_context(tc.tile_pool(name=f"{name}_ps", bufs=2, space="PSUM"))
    nblk = max(1, nsteps // TBLK)
    tblk = min(TBLK, nsteps)
    for blk in range(nblk):
        t0 = blk * tblk
        if mov_is_emb:
            mov = movp.tile([128, kc, tblk, BL], F16)
            nc.sync.dma_start(out=mov[:], in_=mov_dram.ap()[:, :, t0:t0 + tblk, :])
        else:
            mov = movp.tile([128, tblk, kc, BL], F16)
            nc.sync.dma_start(out=mov[:], in_=mov_dram.ap()[:, t0:t0 + tblk, :, :])
        stg = stp.tile([128, tblk, MC, BL], F16)
        for m in range(MC):
            ps = psp.tile([128, tblk, BL], F32)
            for k in range(kc):
                rhs = mov[:, k, :, :] if mov_is_emb else mov[:, :, k, :]
                nc.tensor.matmul(
                    ps[:],
                    w_sb[:, (k * MC + m) * 128:(k * MC + m) * 128 + 128],
                    rhs,
                    start=(k == 0),
                    stop=(k == kc - 1),
                )
            # Evacuate with the (per-partition) bias folded in; cast to fp16.
            nc.scalar.add(stg[:, :, m, :], ps[:], bias_sb[:, m:m + 1])
        nc.sync.dma_start(out=out_dram.ap()[:, t0:t0 + tblk, :, :], in_=stg[:])


def _sphase(tc, nc, ctx, name, zx_dram, w_sb, ident_sb, hinit, nsteps,
            save_dram=None, final_param=None):
    """h[t] = tanh(zx[t] + h[t-1] @ W_hh), transposed layout, fully unrolled."""
    zxp = ctx.enter_context(tc.tile_pool(name=f"{name}_zx", bufs=3))
    hp = ctx.enter_context(tc.tile_pool(name=f"{name}_h", bufs=3))
    psp = ctx.enter_context(tc.tile_pool(name=f"{name}_ps", bufs=2, space="PSUM"))
    hprev = hinit
    sblk = min(SBLK, nsteps)
    for t0 in range(0, nsteps, sblk):
        zxt = zxp.tile([128, sblk, KC, BL], F16)
        nc.sync.dma_start(out=zxt[:], in_=zx_dram.ap()[:, t0:t0 + sblk, :, :])
        for ti in range(sblk):
            t = t0 + ti
            ps = psp.tile([128, KC, BL], F32)
            # Preload zx into PSUM (sets has_written for the whole region).
            nc.tensor.matmul(ps[:], ident_sb[:], zxt[:, ti, :, :],
                             start=True, stop=False)
            for m in range(MC):
                for k in range(KC):
                    c0 = (k * MC + m) * 128
                    nc.tensor.matmul(
                        ps[:, m, :],
                        w_sb[:, c0:c0 + 128],
                        hprev[:, k, :],
                        start=False,
                        stop=(k == KC - 1),
                    )
            if t == nsteps - 1 and final_param is not None:
                fin = hp.tile([128, KC, BL], F32, tag=f"{name}_fin")
                for m in range(MC):
                    nc.scalar.activation(fin[:, m, :], ps[:, m, :], TANH)
                nc.sync.dma_start(out=final_param.ap()[:], in_=fin[:])
            else:
                hnew = hp.tile([128, KC, BL], F16)
                # One tanh per output chunk so next step's early matmuls can
                # start before the last chunk's tanh has finished.
                for m in range(MC):
                    nc.scalar.activation(hnew[:, m, :], ps[:, m, :], TANH)
                if save_dram is not None:
                    nc.sync.dma_start(out=save_dram.ap()[:, t, :, :], in_=hnew[:])
                hprev = hnew


def _build(nsteps):
    from contextlib import ExitStack

    nc = bacc.Bacc("TRN2", target_bir_lowering=False, debug=False,
                   num_devices=NCORES)
    p = nc.declare_dram_parameter
    embT = p("embT", [128, EC, nsteps, BL], F16, False)
    w0i = p("w0i", [128, EC * MC * 128], F16, False)
    w0h = p("w0h", [128, KC * MC * 128], F16, False)
    w1i = p("w1i", [128, KC * MC * 128], F16, False)
    w1h = p("w1h", [128, KC * MC * 128], F16, False)
    zb0 = p("zb0", [128, MC], F32, False)
    zb1 = p("zb1", [128, MC], F32, False)
    ident = p("ident", [128, 128], F16, False)
    hT_out = p("hT_out", [128, KC, BL], F32, True)

    zx0 = nc.dram_tensor("zx0", [128, nsteps, KC, BL], F16)
    h0s = nc.dram_tensor("h0s", [128, nsteps, KC, BL], F16)
    zh1 = nc.dram_tensor("zh1", [128, nsteps, KC, BL], F16)

    with tile.TileContext(nc) as tc, ExitStack() as top:
        wres = top.enter_context(tc.tile_pool(name="wres", bufs=1))
        w0h_sb = wres.tile_from(w0h.ap())
        w1h_sb = wres.tile_from(w1h.ap())
        ident_sb = wres.tile_from(ident.ap())
        zb0_sb = wres.tile_from(zb0.ap())
        zb1_sb = wres.tile_from(zb1.ap())
        hinit = wres.tile([128, KC, BL], F16)
        nc.gpsimd.memset(hinit[:], 0.0)

        with ExitStack() as ctx:
            w0ip = ctx.enter_context(tc.tile_pool(name="w0i", bufs=1))
            w0i_sb = w0ip.tile_from(w0i.ap())
            _pphase(tc, nc, ctx, "p0", w0i_sb, EC, embT, True, zx0, zb0_sb,
                    nsteps)
        tc.strict_bb_all_engine_barrier()
        with ExitStack() as ctx:
            _sphase(tc, nc, ctx, "s1", zx0, w0h_sb, ident_sb, hinit, nsteps,
                    save_dram=h0s)
        tc.strict_bb_all_engine_barrier()
        with ExitStack() as ctx:
            w1ip = ctx.enter_context(tc.tile_pool(name="w1i", bufs=1))
            w1i_sb = w1ip.tile_from(w1i.ap())
            _pphase(tc, nc, ctx, "p1", w1i_sb, KC, h0s, False, zh1, zb1_sb,
                    nsteps)
        tc.strict_bb_all_engine_barrier()
        with ExitStack() as ctx:
            _sphase(tc, nc, ctx, "s2", zh1, w1h_sb, ident_sb, hinit, nsteps,
                    final_param=hT_out)
    nc.compile()
    return nc


def _get_program(nsteps):
    if nsteps not in _programs:
        _programs[nsteps] = _build(nsteps)
    return _programs[nsteps]


def _wchunks(w):
    """[K, H] -> [128, K/128 * 8 * 128] with chunk (k, m) at cols (k*8+m)*128."""
    kcw = w.shape[0] // 128
    return np.ascontiguousarray(
        w.reshape(kcw, 128, MC, 128).transpose(1, 0, 2, 3).reshape(128, -1)
    ).astype(np.float16)


def _bias_cols(b):
    """[H] -> [128, MC] with b[128m+p] at [p, m]."""
    return np.ascontiguousarray(b.reshape(MC, 128).T).astype(np.float32)


def _run(inputs, nsteps):
    global last_results
    inp = {k: np.asarray(v) for k, v in inputs.items()}
    emb_x = inp["emb"].astype(np.float32)[inp["x"]]  # [B, S, E]
    emb_x = emb_x[:, :nsteps]
    ident = np.eye(128, dtype=np.float16)

    in_maps = []
    for c in range(NCORES):
        d = "fw" if c < 4 else "bw"
        b0 = BL * (c % 4)
        seq = emb_x[b0:b0 + BL]                      # [BL, nsteps, E]
        if d == "bw":
            seq = seq[:, ::-1]
        # embT[p, k, t, b] = seq[b, t, 128k+p]
        embT = np.ascontiguousarray(
            seq.transpose(2, 1, 0)                   # [E, t, b]
            .reshape(EC, 128, nsteps, BL)
            .transpose(1, 0, 2, 3)
        ).astype(np.float16)
        in_maps.append({
            "embT": embT,
            "w0i": _wchunks(inp[f"{d}0_wih"]),
            "w0h": _wchunks(inp[f"{d}0_whh"]),
            "w1i": _wchunks(inp[f"{d}1_wih"]),
            "w1h": _wchunks(inp[f"{d}1_whh"]),
            "zb0": _bias_cols(inp[f"{d}0_bih"] + inp[f"{d}0_bhh"]),
            "zb1": _bias_cols(inp[f"{d}1_bih"] + inp[f"{d}1_bhh"]),
            "ident": ident,
        })

    nc = _get_program(nsteps)
    res = run_bass_kernel_spmd(
        nc, in_maps, list(range(NCORES)),
        trace=bool(os.environ.get("BASS_TRACE")),
    )
    last_results = res

    hidden = np.zeros((B, 2 * H), dtype=np.float32)
    for c in range(NCORES):
        out = np.asarray(res.results[c]["hT_out"])   # [128, KC, BL]
        h = out.transpose(1, 0, 2).reshape(H, BL)    # [H, BL]
        b0 = BL * (c % 4)
        if c < 4:
            hidden[b0:b0 + BL, :H] = h.T
        else:
            hidden[b0:b0 + BL, H:] = h.T
    out = (hidden @ inp["fc1_w"].astype(np.float32) + inp["fc1_b"]) \
        @ inp["fc2_w"].astype(np.float32) + inp["fc2_b"]
    return out.astype(np.float32)


def kernel(**inputs):
    return _run(inputs, S)


# revision 5
# speedup vs baseline: 1.5582x; 1.5582x over previous
"""Trainium2 Bass kernel for nn_BiRNN (2-layer bidirectional tanh RNN classifier).

Strategy
--------
The output depends only on the final hidden state of the top layer in each
direction, but the tanh recurrence is strictly sequential in time.  We
restructure the per-direction compute as:

  P0: zx0[t] = emb_x[t] @ W0_ih + (b0_ih + b0_hh)     -- big parallel GEMM
  S1: h0[t]  = tanh(zx0[t] + h0[t-1] @ W0_hh)          -- serial, 512 steps
  P1: zh1[t] = h0[t] @ W1_ih + (b1_ih + b1_hh)         -- big parallel GEMM
  S2: h1[t]  = tanh(zh1[t] + h1[t-1] @ W1_hh)          -- serial, 512 steps

Only the h @ W_hh recurrences stay on the serial critical path.  Everything is
kept in *transposed* layout (hT: [H, B] with H on partitions) so that each
serial step is: stationary = W_hh 128x128 chunks (fp16, fast weight load),
moving = hT chunks, output = next hT directly -- no per-step transposes, and
biases become per-partition scalars folded into the precomputed zx.

The zx[t] term is preloaded into PSUM with an identity-stationary matmul
(start=True), so the 64 accumulating weight matmuls then add onto it; tanh is
applied by ScalarE straight out of PSUM (fp32 internal, 4-ULP table).

Parallelization: per-step collectives are far too slow on this hardware
(multi-us floor), so the two directions run on disjoint cores and the batch is
split 4-ways to shrink the per-core parallel-GEMM phases:
  cores 0-3: forward direction,  batch rows 16c   .. 16c+15
  cores 4-7: backward direction, batch rows 16(c-4) .. 16(c-4)+15
Each core runs the full P0/S1/P1/S2 chain on its shard; no cross-core
communication.  The tiny FC head (8.4 MFLOP) is applied on the host during
unsharding.

Numerics: fp16 operand storage with fp32 PSUM accumulation measures ~1.3e-4
max relative error on the final [64, 2] output vs the fp32 reference.
"""

import os
import sys

import numpy as np

for _p in ("/opt/trn_rl_repo",):
    if _p not in sys.path:
        sys.path.insert(0, _p)

import concourse.bass as bass
import concourse.mybir as mybir
import concourse.tile as tile
from concourse import bacc
from concourse.bass_utils import run_bass_kernel_spmd

# Problem constants (hardcoded per the spec).
B, S, V, E, H, C = 64, 512, 32000, 512, 1024, 2
NCORES = 8
BL = B // 4          # batch rows per core (4-way batch split per direction)
EC = E // 128        # 4   K-chunks for the E-contraction
KC = H // 128        # 8   K-chunks for the H-contraction
MC = H // 128        # 8   output (H) chunks
SBLK = 16            # serial-phase zx streaming block (steps)
TBLK = 32            # parallel-phase block (steps) -> 512 moving columns
F16 = mybir.dt.float16
F32 = mybir.dt.float32
TANH = mybir.ActivationFunctionType.Tanh

_programs: dict = {}   # nsteps -> Bass program
last_results = None    # BassKernelResults of the most recent run (for test.py)


def _pphase(tc, nc, ctx, name, w_sb, kc, mov_dram, mov_is_emb, out_dram, bias_sb,
            nsteps):
    """out[t] = mov[t] @ W + bias, all in transposed/chunked layout."""
    movp = ctx.enter_context(tc.tile_pool(name=f"{name}_mov", bufs=3))
    stp = ctx.enter_context(tc.tile_pool(name=f"{name}_stg", bufs=2))
    psp = ctx.enter_context(tc.tile_pool(name=f"{name}_ps", bufs=2, space="PSUM"))
    nblk = max(1, nsteps // TBLK)
    tblk = min(TBLK, nsteps)
    for blk in range(nblk):
        t0 = blk * tblk
        if mov_is_emb:
            mov = movp.tile([128, kc, tblk, BL], F16)
            nc.sync.dma_start(out=mov[:], in_=mov_dram.ap()[:, :, t0:t0 + tblk, :])
        else:
            mov = movp.tile([128, tblk, kc, BL], F16)
            nc.sync.dma_start(out=mov[:], in_=mov_dram.ap()[:, t0:t0 + tblk, :, :])
        stg = stp.tile([128, tblk, MC, BL], F16)
        for m in range(MC):
            ps = psp.tile([128, tblk, BL], F32)
            for k in range(kc):
                rhs = mov[:, k, :, :] if mov_is_emb else mov[:, :, k, :]
                nc.tensor.matmul(
                    ps[:],
                    w_sb[:, (k * MC + m) * 128:(k * MC + m) * 128 + 128],
                    rhs,
                    start=(k == 0),
                    stop=(k == kc - 1),
                )
            # Evacuate with the (per-partition) bias folded in; cast to fp16.
            nc.scalar.add(stg[:, :, m, :], ps[:], bias_sb[:, m:m + 1])
        nc.sync.dma_start(out=out_dram.ap()[:, t0:t0 + tblk, :, :], in_=stg[:])


def _sphase(tc, nc, ctx, name, zx_dram, w_sb, ident_sb, hinit, nsteps,
            save_dram=None, final_param=None):
    """h[t] = tanh(zx[t] + h[t-1] @ W_hh), transposed layout, fully unrolled."""
    zxp = ctx.enter_context(tc.tile_pool(name=f"{name}_zx", bufs=3))
    hp = ctx.enter_context(tc.tile_pool(name=f"{name}_h", bufs=3))
    psp = ctx.enter_context(tc.tile_pool(name=f"{name}_ps", bufs=2, space="PSUM"))
    hprev = hinit
    sblk = min(SBLK, nsteps)
    for t0 in range(0, nsteps, sblk):
        zxt = zxp.tile([128, sblk, KC, BL], F16)
        nc.sync.dma_start(out=zxt[:], in_=zx_dram.ap()[:, t0:t0 + sblk, :, :])
        for ti in range(sblk):
            t = t0 + ti
            ps = psp.tile([128, KC, BL], F32)
            # Preload zx into PSUM (sets has_written for the whole region).
            nc.tensor.matmul(ps[:], ident_sb[:], zxt[:, ti, :, :],
                             start=True, stop=False)
            for m in range(MC):
                for k in range(KC):
                    c0 = (k * MC + m) * 128
                    nc.tensor.matmul(
                        ps[:, m, :],
                        w_sb[:, c0:c0 + 128],
                        hprev[:, k, :],
                        start=False,
                        stop=(k == KC - 1),
                    )
            if t == nsteps - 1 and final_param is not None:
                fin = hp.tile([128, KC, BL], F32, tag=f"{name}_fin")
                nc.scalar.activation(fin[:], ps[:], TANH)
                nc.sync.dma_start(out=final_param.ap()[:], in_=fin[:])
            else:
                hnew = hp.tile([128, KC, BL], F16)
                # A single tanh instruction: ACT has ~293 ns fixed overhead
                # per instruction, so one [128,128] call (~400 ns) beats
                # eight [128,16] calls (~2.5 us serial chain).
                nc.scalar.activation(hnew[:], ps[:], TANH)
                if save_dram is not None:
                    nc.sync.dma_start(out=save_dram.ap()[:, t, :, :], in_=hnew[:])
                hprev = hnew


def _build(nsteps):
    from contextlib import ExitStack

    nc = bacc.Bacc("TRN2", target_bir_lowering=False, debug=False,
                   num_devices=NCORES)
    p = nc.declare_dram_parameter
    embT = p("embT", [128, EC, nsteps, BL], F16, False)
    w0i = p("w0i", [128, EC * MC * 128], F16, False)
    w0h = p("w0h", [128, KC * MC * 128], F16, False)
    w1i = p("w1i", [128, KC * MC * 128], F16, False)
    w1h = p("w1h", [128, KC * MC * 128], F16, False)
    zb0 = p("zb0", [128, MC], F32, False)
    zb1 = p("zb1", [128, MC], F32, False)
    ident = p("ident", [128, 128], F16, False)
    hT_out = p("hT_out", [128, KC, BL], F32, True)

    zx0 = nc.dram_tensor("zx0", [128, nsteps, KC, BL], F16)
    h0s = nc.dram_tensor("h0s", [128, nsteps, KC, BL], F16)
    zh1 = nc.dram_tensor("zh1", [128, nsteps, KC, BL], F16)

    with tile.TileContext(nc) as tc, ExitStack() as top:
        wres = top.enter_context(tc.tile_pool(name="wres", bufs=1))
        w0h_sb = wres.tile_from(w0h.ap())
        w1h_sb = wres.tile_from(w1h.ap())
        ident_sb = wres.tile_from(ident.ap())
        zb0_sb = wres.tile_from(zb0.ap())
        zb1_sb = wres.tile_from(zb1.ap())
        hinit = wres.tile([128, KC, BL], F16)
        nc.gpsimd.memset(hinit[:], 0.0)

        with ExitStack() as ctx:
            w0ip = ctx.enter_context(tc.tile_pool(name="w0i", bufs=1))
            w0i_sb = w0ip.tile_from(w0i.ap())
            _pphase(tc, nc, ctx, "p0", w0i_sb, EC, embT, True, zx0, zb0_sb,
                    nsteps)
        tc.strict_bb_all_engine_barrier()
        with ExitStack() as ctx:
            _sphase(tc, nc, ctx, "s1", zx0, w0h_sb, ident_sb, hinit, nsteps,
                    save_dram=h0s)
        tc.strict_bb_all_engine_barrier()
        with ExitStack() as ctx:
            w1ip = ctx.enter_context(tc.tile_pool(name="w1i", bufs=1))
            w1i_sb = w1ip.tile_from(w1i.ap())
            _pphase(tc, nc, ctx, "p1", w1i_sb, KC, h0s, False, zh1, zb1_sb,
                    nsteps)
        tc.strict_bb_all_engine_barrier()
        with ExitStack() as ctx:
            _sphase(tc, nc, ctx, "s2", zh1, w1h_sb, ident_sb, hinit, nsteps,
                    final_param=hT_out)
    nc.compile()
    return nc


def _get_program(nsteps):
    if nsteps not in _programs:
        _programs[nsteps] = _build(nsteps)
    return _programs[nsteps]


def _wchunks(w):
    """[K, H] -> [128, K/128 * 8 * 128] with chunk (k, m) at cols (k*8+m)*128."""
    kcw = w.shape[0] // 128
    return np.ascontiguousarray(
        w.reshape(kcw, 128, MC, 128).transpose(1, 0, 2, 3).reshape(128, -1)
    ).astype(np.float16)


def _bias_cols(b):
    """[H] -> [128, MC] with b[128m+p] at [p, m]."""
    return np.ascontiguousarray(b.reshape(MC, 128).T).astype(np.float32)


def _run(inputs, nsteps):
    global last_results
    inp = {k: np.asarray(v) for k, v in inputs.items()}
    emb_x = inp["emb"].astype(np.float32)[inp["x"]]  # [B, S, E]
    emb_x = emb_x[:, :nsteps]
    ident = np.eye(128, dtype=np.float16)

    in_maps = []
    for c in range(NCORES):
        d = "fw" if c < 4 else "bw"
        b0 = BL * (c % 4)
        seq = emb_x[b0:b0 + BL]                      # [BL, nsteps, E]
        if d == "bw":
            seq = seq[:, ::-1]
        # embT[p, k, t, b] = seq[b, t, 128k+p]
        embT = np.ascontiguousarray(
            seq.transpose(2, 1, 0)                   # [E, t, b]
            .reshape(EC, 128, nsteps, BL)
            .transpose(1, 0, 2, 3)
        ).astype(np.float16)
        in_maps.append({
            "embT": embT,
            "w0i": _wchunks(inp[f"{d}0_wih"]),
            "w0h": _wchunks(inp[f"{d}0_whh"]),
            "w1i": _wchunks(inp[f"{d}1_wih"]),
            "w1h": _wchunks(inp[f"{d}1_whh"]),
            "zb0": _bias_cols(inp[f"{d}0_bih"] + inp[f"{d}0_bhh"]),
            "zb1": _bias_cols(inp[f"{d}1_bih"] + inp[f"{d}1_bhh"]),
            "ident": ident,
        })

    nc = _get_program(nsteps)
    res = run_bass_kernel_spmd(
        nc, in_maps, list(range(NCORES)),
        trace=bool(os.environ.get("BASS_TRACE")),
    )
    last_results = res

    hidden = np.zeros((B, 2 * H), dtype=np.float32)
    for c in range(NCORES):
        out = np.asarray(res.results[c]["hT_out"])   # [128, KC, BL]
        h = out.transpose(1, 0, 2).reshape(H, BL)    # [H, BL]
        b0 = BL * (c % 4)
        if c < 4:
            hidden[b0:b0 + BL, :H] = h.T
        else:
            hidden[b0:b0 + BL, H:] = h.T
    out = (hidden @ inp["fc1_w"].astype(np.float32) + inp["fc1_b"]) \
        @ inp["fc2_w"].astype(np.float32) + inp["fc2_b"]
    return out.astype(np.float32)


def kernel(**inputs):
    return _run(inputs, S)


# revision 7
# speedup vs baseline: 1.6044x; 1.0297x over previous
"""Trainium2 Bass kernel for nn_BiRNN (2-layer bidirectional tanh RNN classifier).

Strategy
--------
The output depends only on the final hidden state of the top layer in each
direction, but the tanh recurrence is strictly sequential in time.  We
restructure the per-direction compute as:

  P0: zx0[t] = emb_x[t] @ W0_ih + (b0_ih + b0_hh)     -- big parallel GEMM
  S1: h0[t]  = tanh(zx0[t] + h0[t-1] @ W0_hh)          -- serial, 512 steps
  P1: zh1[t] = h0[t] @ W1_ih + (b1_ih + b1_hh)         -- big parallel GEMM
  S2: h1[t]  = tanh(zh1[t] + h1[t-1] @ W1_hh)          -- serial, 512 steps

Only the h @ W_hh recurrences stay on the serial critical path.  Everything is
kept in *transposed* layout (hT: [H, B] with H on partitions) so that each
serial step is: stationary = W_hh 128x128 chunks (fp16, fast weight load),
moving = hT chunks, output = next hT directly -- no per-step transposes, and
biases become per-partition scalars folded into the precomputed zx.

The zx[t] term is preloaded into PSUM with an identity-stationary matmul
(start=True), so the 64 accumulating weight matmuls then add onto it; tanh is
applied by ScalarE straight out of PSUM (fp32 internal, 4-ULP table).

Parallelization: per-step collectives are far too slow on this hardware
(multi-us floor), so the two directions run on disjoint cores and the batch is
split 4-ways to shrink the per-core parallel-GEMM phases:
  cores 0-3: forward direction,  batch rows 16c   .. 16c+15
  cores 4-7: backward direction, batch rows 16(c-4) .. 16(c-4)+15
Each core runs the full P0/S1/P1/S2 chain on its shard; no cross-core
communication.  The tiny FC head (8.4 MFLOP) is applied on the host during
unsharding.

Numerics: fp16 operand storage with fp32 PSUM accumulation measures ~1.3e-4
max relative error on the final [64, 2] output vs the fp32 reference.
"""

import os
import sys

import numpy as np

for _p in ("/opt/trn_rl_repo",):
    if _p not in sys.path:
        sys.path.insert(0, _p)

import concourse.bass as bass
import concourse.mybir as mybir
import concourse.tile as tile
from concourse import bacc
from concourse.bass_utils import run_bass_kernel_spmd

# Problem constants (hardcoded per the spec).
B, S, V, E, H, C = 64, 512, 32000, 512, 1024, 2
NCORES = 8
BL = B // 4          # batch rows per core (4-way batch split per direction)
EC = E // 128        # 4   K-chunks for the E-contraction
KC = H // 128        # 8   K-chunks for the H-contraction
MC = H // 128        # 8   output (H) chunks
SBLK = 16            # serial-phase zx streaming block (steps)
TBLK = 32            # parallel-phase block (steps) -> 512 moving columns
F16 = mybir.dt.float16
F32 = mybir.dt.float32
TANH = mybir.ActivationFunctionType.Tanh

_programs: dict = {}   # nsteps -> Bass program
last_results = None    # BassKernelResults of the most recent run (for test.py)


def _pphase(tc, nc, ctx, name, w_sb, kc, mov_dram, mov_is_emb, out_dram, bias_sb,
            nsteps):
    """out[t] = mov[t] @ W + bias, all in transposed/chunked layout."""
    movp = ctx.enter_context(tc.tile_pool(name=f"{name}_mov", bufs=3))
    stp = ctx.enter_context(tc.tile_pool(name=f"{name}_stg", bufs=2))
    psp = ctx.enter_context(tc.tile_pool(name=f"{name}_ps", bufs=2, space="PSUM"))
    nblk = max(1, nsteps // TBLK)
    tblk = min(TBLK, nsteps)
    for blk in range(nblk):
        t0 = blk * tblk
        if mov_is_emb:
            mov = movp.tile([128, kc, tblk, BL], F16)
            nc.sync.dma_start(out=mov[:], in_=mov_dram.ap()[:, :, t0:t0 + tblk, :])
        else:
            mov = movp.tile([128, tblk, kc, BL], F16)
            nc.sync.dma_start(out=mov[:], in_=mov_dram.ap()[:, t0:t0 + tblk, :, :])
        stg = stp.tile([128, tblk, MC, BL], F16)
        for m in range(MC):
            ps = psp.tile([128, tblk, BL], F32)
            for k in range(kc):
                rhs = mov[:, k, :, :] if mov_is_emb else mov[:, :, k, :]
                nc.tensor.matmul(
                    ps[:],
                    w_sb[:, (k * MC + m) * 128:(k * MC + m) * 128 + 128],
                    rhs,
                    start=(k == 0),
                    stop=(k == kc - 1),
                )
            # Evacuate with the (per-partition) bias folded in; cast to fp16.
            nc.scalar.add(stg[:, :, m, :], ps[:], bias_sb[:, m:m + 1])
        nc.sync.dma_start(out=out_dram.ap()[:, t0:t0 + tblk, :, :], in_=stg[:])


def _sphase(tc, nc, ctx, name, zx_dram, w_sb, ident_sb, hinit, nsteps,
            save_dram=None, final_param=None):
    """h[t] = tanh(zx[t] + h[t-1] @ W_hh), transposed layout, fully unrolled."""
    zxp = ctx.enter_context(tc.tile_pool(name=f"{name}_zx", bufs=3))
    hp = ctx.enter_context(tc.tile_pool(name=f"{name}_h", bufs=3))
    psp = ctx.enter_context(tc.tile_pool(name=f"{name}_ps", bufs=2, space="PSUM"))
    HC = KC // 2  # chunks per half
    ha, hb = hinit[:, 0:HC, :], hinit[:, HC:KC, :]  # halves of h(t-1)
    sblk = min(SBLK, nsteps)
    for t0 in range(0, nsteps, sblk):
        zxt = zxp.tile([128, sblk, KC, BL], F16)
        nc.sync.dma_start(out=zxt[:], in_=zx_dram.ap()[:, t0:t0 + sblk, :, :])
        for ti in range(sblk):
            t = t0 + ti
            # Two PSUM banks so the first tanh (half A) can run while the
            # tensor engine is still accumulating into half B.
            psA = psp.tile([128, HC, BL], F32, tag=f"{name}_psA")
            psB = psp.tile([128, HC, BL], F32, tag=f"{name}_psB")
            # Preload zx (sets has_written; independent of h(t-1), so these
            # issue during the previous step's tanh latency).
            nc.tensor.matmul(psA[:], ident_sb[:], zxt[:, ti, 0:HC, :],
                             start=True, stop=False)
            nc.tensor.matmul(psB[:], ident_sb[:], zxt[:, ti, HC:KC, :],
                             start=True, stop=False)
            # k-outer: the first 32 matmuls consume only half A of h(t-1),
            # so the sem-latency + tanh of half B hides under them.
            for k in range(KC):
                rhs = ha[:, k, :] if k < HC else hb[:, k - HC, :]
                for m in range(MC):
                    tgt = psA[:, m, :] if m < HC else psB[:, m - HC, :]
                    c0 = (k * MC + m) * 128
                    nc.tensor.matmul(tgt, w_sb[:, c0:c0 + 128], rhs,
                                     start=False, stop=(k == KC - 1))
            if t == nsteps - 1 and final_param is not None:
                finA = hp.tile([128, HC, BL], F32, tag=f"{name}_finA")
                finB = hp.tile([128, HC, BL], F32, tag=f"{name}_finB")
                nc.scalar.activation(finA[:], psA[:], TANH)
                nc.scalar.activation(finB[:], psB[:], TANH)
                nc.sync.dma_start(out=final_param.ap()[:, 0:HC, :], in_=finA[:])
                nc.sync.dma_start(out=final_param.ap()[:, HC:KC, :], in_=finB[:])
            else:
                # Separate tiles per half so consumers of half A never wait
                # on half B's tanh.
                hna = hp.tile([128, HC, BL], F16, tag=f"{name}_hA")
                hnb = hp.tile([128, HC, BL], F16, tag=f"{name}_hB")
                nc.scalar.activation(hna[:], psA[:], TANH)
                nc.scalar.activation(hnb[:], psB[:], TANH)
                if save_dram is not None:
                    nc.sync.dma_start(out=save_dram.ap()[:, t, 0:HC, :],
                                      in_=hna[:])
                    nc.sync.dma_start(out=save_dram.ap()[:, t, HC:KC, :],
                                      in_=hnb[:])
                ha, hb = hna, hnb


def _build(nsteps):
    from contextlib import ExitStack

    nc = bacc.Bacc("TRN2", target_bir_lowering=False, debug=False,
                   num_devices=NCORES)
    p = nc.declare_dram_parameter
    embT = p("embT", [128, EC, nsteps, BL], F16, False)
    w0i = p("w0i", [128, EC * MC * 128], F16, False)
    w0h = p("w0h", [128, KC * MC * 128], F16, False)
    w1i = p("w1i", [128, KC * MC * 128], F16, False)
    w1h = p("w1h", [128, KC * MC * 128], F16, False)
    zb0 = p("zb0", [128, MC], F32, False)
    zb1 = p("zb1", [128, MC], F32, False)
    ident = p("ident", [128, 128], F16, False)
    hT_out = p("hT_out", [128, KC, BL], F32, True)

    zx0 = nc.dram_tensor("zx0", [128, nsteps, KC, BL], F16)
    h0s = nc.dram_tensor("h0s", [128, nsteps, KC, BL], F16)
    zh1 = nc.dram_tensor("zh1", [128, nsteps, KC, BL], F16)

    with tile.TileContext(nc) as tc, ExitStack() as top:
        wres = top.enter_context(tc.tile_pool(name="wres", bufs=1))
        w0h_sb = wres.tile_from(w0h.ap())
        w1h_sb = wres.tile_from(w1h.ap())
        ident_sb = wres.tile_from(ident.ap())
        zb0_sb = wres.tile_from(zb0.ap())
        zb1_sb = wres.tile_from(zb1.ap())
        hinit = wres.tile([128, KC, BL], F16)
        nc.gpsimd.memset(hinit[:], 0.0)

        with ExitStack() as ctx:
            w0ip = ctx.enter_context(tc.tile_pool(name="w0i", bufs=1))
            w0i_sb = w0ip.tile_from(w0i.ap())
            _pphase(tc, nc, ctx, "p0", w0i_sb, EC, embT, True, zx0, zb0_sb,
                    nsteps)
        tc.strict_bb_all_engine_barrier()
        with ExitStack() as ctx:
            _sphase(tc, nc, ctx, "s1", zx0, w0h_sb, ident_sb, hinit, nsteps,
                    save_dram=h0s)
        tc.strict_bb_all_engine_barrier()
        with ExitStack() as ctx:
            w1ip = ctx.enter_context(tc.tile_pool(name="w1i", bufs=1))
            w1i_sb = w1ip.tile_from(w1i.ap())
            _pphase(tc, nc, ctx, "p1", w1i_sb, KC, h0s, False, zh1, zb1_sb,
                    nsteps)
        tc.strict_bb_all_engine_barrier()
        with ExitStack() as ctx:
            _sphase(tc, nc, ctx, "s2", zh1, w1h_sb, ident_sb, hinit, nsteps,
                    final_param=hT_out)
    nc.compile()
    return nc


def _get_program(nsteps):
    if nsteps not in _programs:
        _programs[nsteps] = _build(nsteps)
    return _programs[nsteps]


def _wchunks(w):
    """[K, H] -> [128, K/128 * 8 * 128] with chunk (k, m) at cols (k*8+m)*128."""
    kcw = w.shape[0] // 128
    return np.ascontiguousarray(
        w.reshape(kcw, 128, MC, 128).transpose(1, 0, 2, 3).reshape(128, -1)
    ).astype(np.float16)


def _bias_cols(b):
    """[H] -> [128, MC] with b[128m+p] at [p, m]."""
    return np.ascontiguousarray(b.reshape(MC, 128).T).astype(np.float32)


def _run(inputs, nsteps):
    global last_results
    inp = {k: np.asarray(v) for k, v in inputs.items()}
    emb_x = inp["emb"].astype(np.float32)[inp["x"]]  # [B, S, E]
    emb_x = emb_x[:, :nsteps]
    ident = np.eye(128, dtype=np.float16)

    in_maps = []
    for c in range(NCORES):
        d = "fw" if c < 4 else "bw"
        b0 = BL * (c % 4)
        seq = emb_x[b0:b0 + BL]                      # [BL, nsteps, E]
        if d == "bw":
            seq = seq[:, ::-1]
        # embT[p, k, t, b] = seq[b, t, 128k+p]
        embT = np.ascontiguousarray(
            seq.transpose(2, 1, 0)                   # [E, t, b]
            .reshape(EC, 128, nsteps, BL)
            .transpose(1, 0, 2, 3)
        ).astype(np.float16)
        in_maps.append({
            "embT": embT,
            "w0i": _wchunks(inp[f"{d}0_wih"]),
            "w0h": _wchunks(inp[f"{d}0_whh"]),
            "w1i": _wchunks(inp[f"{d}1_wih"]),
            "w1h": _wchunks(inp[f"{d}1_whh"]),
            "zb0": _bias_cols(inp[f"{d}0_bih"] + inp[f"{d}0_bhh"]),
            "zb1": _bias_cols(inp[f"{d}1_bih"] + inp[f"{d}1_bhh"]),
            "ident": ident,
        })

    nc = _get_program(nsteps)
    res = run_bass_kernel_spmd(
        nc, in_maps, list(range(NCORES)),
        trace=bool(os.environ.get("BASS_TRACE")),
    )
    last_results = res

    hidden = np.zeros((B, 2 * H), dtype=np.float32)
    for c in range(NCORES):
        out = np.asarray(res.results[c]["hT_out"])   # [128, KC, BL]
        h = out.transpose(1, 0, 2).reshape(H, BL)    # [H, BL]
        b0 = BL * (c % 4)
        if c < 4:
            hidden[b0:b0 + BL, :H] = h.T
        else:
            hidden[b0:b0 + BL, H:] = h.T
    out = (hidden @ inp["fc1_w"].astype(np.float32) + inp["fc1_b"]) \
        @ inp["fc2_w"].astype(np.float32) + inp["fc2_b"]
    return out.astype(np.float32)


def kernel(**inputs):
    return _run(inputs, S)


# revision 11
# speedup vs baseline: 1.8687x; 1.1647x over previous
"""Trainium2 Bass kernel for nn_BiRNN (2-layer bidirectional tanh RNN classifier).

Strategy
--------
The output depends only on the final hidden state of the top layer in each
direction, but the tanh recurrence is strictly sequential in time.  We
restructure the per-direction compute as:

  P0: zx0[t] = emb_x[t] @ W0_ih + (b0_ih + b0_hh)     -- big parallel GEMM
  S1: h0[t]  = tanh(zx0[t] + h0[t-1] @ W0_hh)          -- serial, 512 steps
  P1: zh1[t] = h0[t] @ W1_ih + (b1_ih + b1_hh)         -- big parallel GEMM
  S2: h1[t]  = tanh(zh1[t] + h1[t-1] @ W1_hh)          -- serial, 512 steps

Only the h @ W_hh recurrences stay on the serial critical path.  Everything is
kept in *transposed* layout (hT: [H, B] with H on partitions) so that each
serial step is: stationary = W_hh 128x128 chunks (fp16, fast weight load),
moving = hT chunks, output = next hT directly -- no per-step transposes, and
biases become per-partition scalars folded into the precomputed zx.

The zx[t] term is preloaded into PSUM with an identity-stationary matmul
(start=True), so the 64 accumulating weight matmuls then add onto it; tanh is
applied by ScalarE straight out of PSUM (fp32 internal, 4-ULP table).

Parallelization: per-step collectives are far too slow on this hardware
(multi-us floor), so the two directions run on disjoint cores and the batch is
split 4-ways to shrink the per-core parallel-GEMM phases:
  cores 0-3: forward direction,  batch rows 16c   .. 16c+15
  cores 4-7: backward direction, batch rows 16(c-4) .. 16(c-4)+15
Each core runs the full P0/S1/P1/S2 chain on its shard; no cross-core
communication.  The tiny FC head (8.4 MFLOP) is applied on the host during
unsharding.

Numerics: fp16 operand storage with fp32 PSUM accumulation measures ~1.3e-4
max relative error on the final [64, 2] output vs the fp32 reference.
"""

import os
import sys

import numpy as np

for _p in ("/opt/trn_rl_repo",):
    if _p not in sys.path:
        sys.path.insert(0, _p)

import concourse.bass as bass
import concourse.mybir as mybir
import concourse.tile as tile
from concourse import bacc
from concourse.bass_utils import run_bass_kernel_spmd

# Problem constants (hardcoded per the spec).
B, S, V, E, H, C = 64, 512, 32000, 512, 1024, 2
NCORES = 8
BL = B // 4          # batch rows per core (4-way batch split per direction)
EC = E // 128        # 4   K-chunks for the E-contraction
KC = H // 128        # 8   K-chunks for the H-contraction
MC = H // 128        # 8   output (H) chunks
SBLK = 16            # serial-phase zx streaming block (steps)
TBLK = 32            # parallel-phase block (steps) -> 512 moving columns
F16 = mybir.dt.float16
F32 = mybir.dt.float32
TANH = mybir.ActivationFunctionType.Tanh

_programs: dict = {}   # nsteps -> Bass program
last_results = None    # BassKernelResults of the most recent run (for test.py)


def _pphase(tc, nc, ctx, name, w_sb, kc, mov_dram, mov_is_emb, out_dram, bias_sb,
            nsteps):
    """out[t] = mov[t] @ W + bias, all in transposed/chunked layout."""
    movp = ctx.enter_context(tc.tile_pool(name=f"{name}_mov", bufs=3))
    stp = ctx.enter_context(tc.tile_pool(name=f"{name}_stg", bufs=2))
    psp = ctx.enter_context(tc.tile_pool(name=f"{name}_ps", bufs=2, space="PSUM"))
    nblk = max(1, nsteps // TBLK)
    tblk = min(TBLK, nsteps)
    for blk in range(nblk):
        t0 = blk * tblk
        if mov_is_emb:
            mov = movp.tile([128, kc, tblk, BL], F16)
            nc.sync.dma_start(out=mov[:], in_=mov_dram.ap()[:, :, t0:t0 + tblk, :])
        else:
            mov = movp.tile([128, tblk, kc, BL], F16)
            nc.sync.dma_start(out=mov[:], in_=mov_dram.ap()[:, t0:t0 + tblk, :, :])
        stg = stp.tile([128, tblk, MC, BL], F16)
        for m in range(MC):
            ps = psp.tile([128, tblk, BL], F32)
            for k in range(kc):
                rhs = mov[:, k, :, :] if mov_is_emb else mov[:, :, k, :]
                nc.tensor.matmul(
                    ps[:],
                    w_sb[:, (k * MC + m) * 128:(k * MC + m) * 128 + 128],
                    rhs,
                    start=(k == 0),
                    stop=(k == kc - 1),
                )
            # Evacuate with the (per-partition) bias folded in; cast to fp16.
            nc.scalar.add(stg[:, :, m, :], ps[:], bias_sb[:, m:m + 1])
        nc.sync.dma_start(out=out_dram.ap()[:, t0:t0 + tblk, :, :], in_=stg[:])


def _fused(tc, nc, ctx, zx_dram, w0h_sb, w1h_sb, w1i_sb, zb1_sb, ident_sb,
           hinit, nsteps, final_param):
    """Interleaved S1 (h0 recurrence), inline P1 blocks, and S2 (h1
    recurrence), all on one core.  S1 and S2 are independent dependency
    chains, so each one's tanh/sem latency hides under the other's matmuls.
    h0 history and zh1 live in SBUF rings; nothing round-trips through HBM.
    """
    TB = min(TBLK, nsteps)          # P1 block size (steps)
    WIN = 3 * TB                    # ring slots (3 blocks)
    LAG = TB + 1                    # S2 trails S1 by this many steps
    HC = KC // 2

    zxp = ctx.enter_context(tc.tile_pool(name="f_zx", bufs=3))
    ringp = ctx.enter_context(tc.tile_pool(name="f_ring", bufs=1))
    hp = ctx.enter_context(tc.tile_pool(name="f_h", bufs=3))
    psp = ctx.enter_context(tc.tile_pool(name="f_ps", bufs=2, space="PSUM"))
    psp2 = ctx.enter_context(tc.tile_pool(name="f_ps2", bufs=1, space="PSUM"))
    p1psp = ctx.enter_context(tc.tile_pool(name="f_p1ps", bufs=2, space="PSUM"))

    hwin = ringp.tile([128, WIN, KC, BL], F16)   # h0 history ring
    zwin = ringp.tile([128, WIN, MC, BL], F16)   # zh1 ring

    state = {
        "s1": (hinit[:, 0:HC, :], hinit[:, HC:KC, :]),
        "s2": (hinit[:, 0:HC, :], hinit[:, HC:KC, :]),
        "zxt": None,
    }

    def rnn_step(which, t, w_sb_, zx_a, zx_b, out_a, out_b):
        """One recurrence step: psum = zx + W_hh^T h(t-1); out = tanh(psum)."""
        ha, hb = state[which]
        pool = psp if which == "s1" else psp2
        psA = pool.tile([128, HC, BL], F32, tag=f"{which}_psA")
        psB = pool.tile([128, HC, BL], F32, tag=f"{which}_psB")
        nc.tensor.matmul(psA[:], ident_sb[:], zx_a, start=True, stop=False)
        nc.tensor.matmul(psB[:], ident_sb[:], zx_b, start=True, stop=False)
        for k in range(KC):
            rhs = ha[:, k, :] if k < HC else hb[:, k - HC, :]
            for m in range(MC):
                tgt = psA[:, m, :] if m < HC else psB[:, m - HC, :]
                c0 = (k * MC + m) * 128
                nc.tensor.matmul(tgt, w_sb_[:, c0:c0 + 128], rhs,
                                 start=False, stop=(k == KC - 1))
        nc.scalar.activation(out_a, psA[:], TANH)
        nc.scalar.activation(out_b, psB[:], TANH)

    for t in range(nsteps + LAG):
        if t < nsteps:
            # ---- S1 step t ----
            if t % SBLK == 0:
                sblk = min(SBLK, nsteps - t)
                zxt = zxp.tile([128, sblk, KC, BL], F16, tag="f_zxt")
                nc.sync.dma_start(out=zxt[:],
                                  in_=zx_dram.ap()[:, t:t + sblk, :, :])
                state["zxt"] = zxt
            zxt = state["zxt"]
            ti = t % SBLK
            s = t % WIN
            rnn_step("s1", t, w0h_sb,
                     zxt[:, ti, 0:HC, :], zxt[:, ti, HC:KC, :],
                     hwin[:, s, 0:HC, :], hwin[:, s, HC:KC, :])
            state["s1"] = (hwin[:, s, 0:HC, :], hwin[:, s, HC:KC, :])
            # ---- inline P1 block once its h0 inputs are complete ----
            if t % TB == TB - 1:
                b = t // TB
                s0 = TB * (b % 3)
                for m in range(MC):
                    ps = p1psp.tile([128, TB, BL], F32, tag="p1_ps")
                    for k in range(KC):
                        c0 = (k * MC + m) * 128
                        nc.tensor.matmul(ps[:], w1i_sb[:, c0:c0 + 128],
                                         hwin[:, s0:s0 + TB, k, :],
                                         start=(k == 0), stop=(k == KC - 1))
                    nc.scalar.add(zwin[:, s0:s0 + TB, m, :], ps[:],
                                  zb1_sb[:, m:m + 1])
        u = t - LAG
        if 0 <= u < nsteps:
            # ---- S2 step u ----
            su = u % WIN
            if u == nsteps - 1:
                finA = hp.tile([128, HC, BL], F32, tag="finA")
                finB = hp.tile([128, HC, BL], F32, tag="finB")
                rnn_step("s2", u, w1h_sb,
                         zwin[:, su, 0:HC, :], zwin[:, su, HC:KC, :],
                         finA[:], finB[:])
                nc.sync.dma_start(out=final_param.ap()[:, 0:HC, :],
                                  in_=finA[:])
                nc.sync.dma_start(out=final_param.ap()[:, HC:KC, :],
                                  in_=finB[:])
            else:
                hna = hp.tile([128, HC, BL], F16, tag="s2_hA")
                hnb = hp.tile([128, HC, BL], F16, tag="s2_hB")
                rnn_step("s2", u, w1h_sb,
                         zwin[:, su, 0:HC, :], zwin[:, su, HC:KC, :],
                         hna[:], hnb[:])
                state["s2"] = (hna, hnb)


def _sphase(tc, nc, ctx, name, zx_dram, w_sb, ident_sb, hinit, nsteps,
            save_dram=None, final_param=None):
    """h[t] = tanh(zx[t] + h[t-1] @ W_hh), transposed layout, fully unrolled."""
    zxp = ctx.enter_context(tc.tile_pool(name=f"{name}_zx", bufs=3))
    hp = ctx.enter_context(tc.tile_pool(name=f"{name}_h", bufs=3))
    psp = ctx.enter_context(tc.tile_pool(name=f"{name}_ps", bufs=2, space="PSUM"))
    HC = KC // 2  # chunks per half
    ha, hb = hinit[:, 0:HC, :], hinit[:, HC:KC, :]  # halves of h(t-1)
    sblk = min(SBLK, nsteps)
    for t0 in range(0, nsteps, sblk):
        zxt = zxp.tile([128, sblk, KC, BL], F16)
        nc.sync.dma_start(out=zxt[:], in_=zx_dram.ap()[:, t0:t0 + sblk, :, :])
        for ti in range(sblk):
            t = t0 + ti
            # Two PSUM banks so the first tanh (half A) can run while the
            # tensor engine is still accumulating into half B.
            psA = psp.tile([128, HC, BL], F32, tag=f"{name}_psA")
            psB = psp.tile([128, HC, BL], F32, tag=f"{name}_psB")
            # Preload zx (sets has_written; independent of h(t-1), so these
            # issue during the previous step's tanh latency).
            nc.tensor.matmul(psA[:], ident_sb[:], zxt[:, ti, 0:HC, :],
                             start=True, stop=False)
            nc.tensor.matmul(psB[:], ident_sb[:], zxt[:, ti, HC:KC, :],
                             start=True, stop=False)
            # k-outer: the first 32 matmuls consume only half A of h(t-1),
            # so the sem-latency + tanh of half B hides under them.
            for k in range(KC):
                rhs = ha[:, k, :] if k < HC else hb[:, k - HC, :]
                for m in range(MC):
                    tgt = psA[:, m, :] if m < HC else psB[:, m - HC, :]
                    c0 = (k * MC + m) * 128
                    nc.tensor.matmul(tgt, w_sb[:, c0:c0 + 128], rhs,
                                     start=False, stop=(k == KC - 1))
            if t == nsteps - 1 and final_param is not None:
                finA = hp.tile([128, HC, BL], F32, tag=f"{name}_finA")
                finB = hp.tile([128, HC, BL], F32, tag=f"{name}_finB")
                nc.scalar.activation(finA[:], psA[:], TANH)
                nc.scalar.activation(finB[:], psB[:], TANH)
                nc.sync.dma_start(out=final_param.ap()[:, 0:HC, :], in_=finA[:])
                nc.sync.dma_start(out=final_param.ap()[:, HC:KC, :], in_=finB[:])
            else:
                # Separate tiles per half so consumers of half A never wait
                # on half B's tanh.
                hna = hp.tile([128, HC, BL], F16, tag=f"{name}_hA")
                hnb = hp.tile([128, HC, BL], F16, tag=f"{name}_hB")
                nc.scalar.activation(hna[:], psA[:], TANH)
                nc.scalar.activation(hnb[:], psB[:], TANH)
                if save_dram is not None:
                    nc.sync.dma_start(out=save_dram.ap()[:, t, 0:HC, :],
                                      in_=hna[:])
                    nc.sync.dma_start(out=save_dram.ap()[:, t, HC:KC, :],
                                      in_=hnb[:])
                ha, hb = hna, hnb


def _build(nsteps):
    from contextlib import ExitStack

    nc = bacc.Bacc("TRN2", target_bir_lowering=False, debug=False,
                   num_devices=NCORES)
    p = nc.declare_dram_parameter
    embT = p("embT", [128, EC, nsteps, BL], F16, False)
    w0i = p("w0i", [128, EC * MC * 128], F16, False)
    w0h = p("w0h", [128, KC * MC * 128], F16, False)
    w1i = p("w1i", [128, KC * MC * 128], F16, False)
    w1h = p("w1h", [128, KC * MC * 128], F16, False)
    zb0 = p("zb0", [128, MC], F32, False)
    zb1 = p("zb1", [128, MC], F32, False)
    ident = p("ident", [128, 128], F16, False)
    hT_out = p("hT_out", [128, KC, BL], F32, True)

    zx0 = nc.dram_tensor("zx0", [128, nsteps, KC, BL], F16)

    with tile.TileContext(nc) as tc, ExitStack() as top:
        wres = top.enter_context(tc.tile_pool(name="wres", bufs=1))
        w0h_sb = wres.tile_from(w0h.ap())
        w1h_sb = wres.tile_from(w1h.ap())
        w1i_sb = wres.tile_from(w1i.ap())
        ident_sb = wres.tile_from(ident.ap())
        zb0_sb = wres.tile_from(zb0.ap())
        zb1_sb = wres.tile_from(zb1.ap())
        hinit = wres.tile([128, KC, BL], F16)
        nc.gpsimd.memset(hinit[:], 0.0)

        with ExitStack() as ctx:
            w0ip = ctx.enter_context(tc.tile_pool(name="w0i", bufs=1))
            w0i_sb = w0ip.tile_from(w0i.ap())
            _pphase(tc, nc, ctx, "p0", w0i_sb, EC, embT, True, zx0, zb0_sb,
                    nsteps)
        tc.strict_bb_all_engine_barrier()
        with ExitStack() as ctx:
            _fused(tc, nc, ctx, zx0, w0h_sb, w1h_sb, w1i_sb, zb1_sb,
                   ident_sb, hinit, nsteps, hT_out)
    nc.compile()
    return nc


def _get_program(nsteps):
    if nsteps not in _programs:
        _programs[nsteps] = _build(nsteps)
    return _programs[nsteps]


def _wchunks(w):
    """[K, H] -> [128, K/128 * 8 * 128] with chunk (k, m) at cols (k*8+m)*128."""
    kcw = w.shape[0] // 128
    return np.ascontiguousarray(
        w.reshape(kcw, 128, MC, 128).transpose(1, 0, 2, 3).reshape(128, -1)
    ).astype(np.float16)


def _bias_cols(b):
    """[H] -> [128, MC] with b[128m+p] at [p, m]."""
    return np.ascontiguousarray(b.reshape(MC, 128).T).astype(np.float32)


def _run(inputs, nsteps):
    global last_results
    inp = {k: np.asarray(v) for k, v in inputs.items()}
    emb_x = inp["emb"].astype(np.float32)[inp["x"]]  # [B, S, E]
    emb_x = emb_x[:, :nsteps]
    ident = np.eye(128, dtype=np.float16)

    in_maps = []
    for c in range(NCORES):
        d = "fw" if c < 4 else "bw"
        b0 = BL * (c % 4)
        seq = emb_x[b0:b0 + BL]                      # [BL, nsteps, E]
        if d == "bw":
            seq = seq[:, ::-1]
        # embT[p, k, t, b] = seq[b, t, 128k+p]
        embT = np.ascontiguousarray(
            seq.transpose(2, 1, 0)                   # [E, t, b]
            .reshape(EC, 128, nsteps, BL)
            .transpose(1, 0, 2, 3)
        ).astype(np.float16)
        in_maps.append({
            "embT": embT,
            "w0i": _wchunks(inp[f"{d}0_wih"]),
            "w0h": _wchunks(inp[f"{d}0_whh"]),
            "w1i": _wchunks(inp[f"{d}1_wih"]),
            "w1h": _wchunks(inp[f"{d}1_whh"]),
            "zb0": _bias_cols(inp[f"{d}0_bih"] + inp[f"{d}0_bhh"]),
            "zb1": _bias_cols(inp[f"{d}1_bih"] + inp[f"{d}1_bhh"]),
            "ident": ident,
        })

    nc = _get_program(nsteps)
    res = run_bass_kernel_spmd(
        nc, in_maps, list(range(NCORES)),
        trace=bool(os.environ.get("BASS_TRACE")),
    )
    last_results = res

    hidden = np.zeros((B, 2 * H), dtype=np.float32)
    for c in range(NCORES):
        out = np.asarray(res.results[c]["hT_out"])   # [128, KC, BL]
        h = out.transpose(1, 0, 2).reshape(H, BL)    # [H, BL]
        b0 = BL * (c % 4)
        if c < 4:
            hidden[b0:b0 + BL, :H] = h.T
        else:
            hidden[b0:b0 + BL, H:] = h.T
    out = (hidden @ inp["fc1_w"].astype(np.float32) + inp["fc1_b"]) \
        @ inp["fc2_w"].astype(np.float32) + inp["fc2_b"]
    return out.astype(np.float32)


def kernel(**inputs):
    return _run(inputs, S)


# revision 12
# speedup vs baseline: 1.8770x; 1.0044x over previous
"""Trainium2 Bass kernel for nn_BiRNN (2-layer bidirectional tanh RNN classifier).

Strategy
--------
The output depends only on the final hidden state of the top layer in each
direction, but the tanh recurrence is strictly sequential in time.  We
restructure the per-direction compute as:

  P0: zx0[t] = emb_x[t] @ W0_ih + (b0_ih + b0_hh)     -- big parallel GEMM
  S1: h0[t]  = tanh(zx0[t] + h0[t-1] @ W0_hh)          -- serial, 512 steps
  P1: zh1[t] = h0[t] @ W1_ih + (b1_ih + b1_hh)         -- big parallel GEMM
  S2: h1[t]  = tanh(zh1[t] + h1[t-1] @ W1_hh)          -- serial, 512 steps

Only the h @ W_hh recurrences stay on the serial critical path.  Everything is
kept in *transposed* layout (hT: [H, B] with H on partitions) so that each
serial step is: stationary = W_hh 128x128 chunks (fp16, fast weight load),
moving = hT chunks, output = next hT directly -- no per-step transposes, and
biases become per-partition scalars folded into the precomputed zx.

The zx[t] term is preloaded into PSUM with an identity-stationary matmul
(start=True), so the 64 accumulating weight matmuls then add onto it; tanh is
applied by ScalarE straight out of PSUM (fp32 internal, 4-ULP table).

Parallelization: per-step collectives are far too slow on this hardware
(multi-us floor), so the two directions run on disjoint cores and the batch is
split 4-ways to shrink the per-core parallel-GEMM phases:
  cores 0-3: forward direction,  batch rows 16c   .. 16c+15
  cores 4-7: backward direction, batch rows 16(c-4) .. 16(c-4)+15
Each core runs the full P0/S1/P1/S2 chain on its shard; no cross-core
communication.  The tiny FC head (8.4 MFLOP) is applied on the host during
unsharding.

Numerics: fp16 operand storage with fp32 PSUM accumulation measures ~1.3e-4
max relative error on the final [64, 2] output vs the fp32 reference.
"""

import os
import sys

import numpy as np

for _p in ("/opt/trn_rl_repo",):
    if _p not in sys.path:
        sys.path.insert(0, _p)

import concourse.bass as bass
import concourse.mybir as mybir
import concourse.tile as tile
from concourse import bacc
from concourse.bass_utils import run_bass_kernel_spmd

# Problem constants (hardcoded per the spec).
B, S, V, E, H, C = 64, 512, 32000, 512, 1024, 2
NCORES = 8
BL = B // 4          # batch rows per core (4-way batch split per direction)
EC = E // 128        # 4   K-chunks for the E-contraction
KC = H // 128        # 8   K-chunks for the H-contraction
MC = H // 128        # 8   output (H) chunks
BW = 32              # recurrence moving width: BL real + garbage lanes
SBLK = 16            # serial-phase zx streaming block (steps)
TBLK = 32            # parallel-phase block (steps) -> 512 moving columns
F16 = mybir.dt.float16
F32 = mybir.dt.float32
TANH = mybir.ActivationFunctionType.Tanh

_programs: dict = {}   # nsteps -> Bass program
last_results = None    # BassKernelResults of the most recent run (for test.py)


def _pphase(tc, nc, ctx, name, w_sb, kc, mov_dram, mov_is_emb, out_dram, bias_sb,
            nsteps):
    """out[t] = mov[t] @ W + bias, all in transposed/chunked layout."""
    movp = ctx.enter_context(tc.tile_pool(name=f"{name}_mov", bufs=3))
    stp = ctx.enter_context(tc.tile_pool(name=f"{name}_stg", bufs=2))
    psp = ctx.enter_context(tc.tile_pool(name=f"{name}_ps", bufs=2, space="PSUM"))
    nblk = max(1, nsteps // TBLK)
    tblk = min(TBLK, nsteps)
    for blk in range(nblk):
        t0 = blk * tblk
        if mov_is_emb:
            mov = movp.tile([128, kc, tblk, BL], F16)
            nc.sync.dma_start(out=mov[:], in_=mov_dram.ap()[:, :, t0:t0 + tblk, :])
        else:
            mov = movp.tile([128, tblk, kc, BL], F16)
            nc.sync.dma_start(out=mov[:], in_=mov_dram.ap()[:, t0:t0 + tblk, :, :])
        stg = stp.tile([128, tblk, MC, BL], F16)
        for m in range(MC):
            ps = psp.tile([128, tblk, BL], F32)
            for k in range(kc):
                rhs = mov[:, k, :, :] if mov_is_emb else mov[:, :, k, :]
                nc.tensor.matmul(
                    ps[:],
                    w_sb[:, (k * MC + m) * 128:(k * MC + m) * 128 + 128],
                    rhs,
                    start=(k == 0),
                    stop=(k == kc - 1),
                )
            # Evacuate with the (per-partition) bias folded in; cast to fp16.
            nc.scalar.add(stg[:, :, m, :], ps[:], bias_sb[:, m:m + 1])
        nc.sync.dma_start(out=out_dram.ap()[:, t0:t0 + tblk, :, :], in_=stg[:])


def _fused(tc, nc, ctx, zx_dram, w0h_sb, w1h_sb, w1i_sb, zb1_sb, ident_sb,
           hinit, nsteps, final_param):
    """Interleaved S1 (h0 recurrence), inline P1 blocks, and S2 (h1
    recurrence), all on one core.  S1 and S2 are independent dependency
    chains, so each one's tanh/sem latency hides under the other's matmuls.
    h0 history and zh1 live in SBUF rings; nothing round-trips through HBM.
    """
    TB = min(TBLK, nsteps)          # P1 block size (steps)
    WIN = 3 * TB                    # ring slots (3 blocks)
    LAG = TB + 1                    # S2 trails S1 by this many steps
    HC = KC // 2

    zxp = ctx.enter_context(tc.tile_pool(name="f_zx", bufs=3))
    ringp = ctx.enter_context(tc.tile_pool(name="f_ring", bufs=1))
    hp = ctx.enter_context(tc.tile_pool(name="f_h", bufs=3))
    psp = ctx.enter_context(tc.tile_pool(name="f_ps", bufs=2, space="PSUM"))
    psp2 = ctx.enter_context(tc.tile_pool(name="f_ps2", bufs=1, space="PSUM"))
    p1psp = ctx.enter_context(tc.tile_pool(name="f_p1ps", bufs=2, space="PSUM"))

    hwin = ringp.tile([128, WIN, KC, BW], F16)   # h0 history ring (wide)
    zwin = ringp.tile([128, WIN, MC, BL], F16)   # zh1 ring

    state = {
        "s1": (hinit[:, 0:HC, :], hinit[:, HC:KC, :]),
        "s2": (hinit[:, 0:HC, :], hinit[:, HC:KC, :]),
        "zxt": None,
    }

    def rnn_step(which, t, w_sb_, zx_a, zx_b, out_a, out_b):
        """One recurrence step: psum = zx + W_hh^T h(t-1); out = tanh(psum)."""
        ha, hb = state[which]
        pool = psp if which == "s1" else psp2
        psA = pool.tile([128, HC, BW], F32, tag=f"{which}_psA")
        psB = pool.tile([128, HC, BW], F32, tag=f"{which}_psB")
        nc.tensor.matmul(psA[:, :, 0:BL], ident_sb[:], zx_a,
                         start=True, stop=False)
        nc.tensor.matmul(psB[:, :, 0:BL], ident_sb[:], zx_b,
                         start=True, stop=False)
        for k in range(KC):
            rhs = ha[:, k, :] if k < HC else hb[:, k - HC, :]
            for m in range(MC):
                tgt = psA[:, m, :] if m < HC else psB[:, m - HC, :]
                c0 = (k * MC + m) * 128
                nc.tensor.matmul(tgt, w_sb_[:, c0:c0 + 128], rhs,
                                 start=False, stop=(k == KC - 1))
        nc.scalar.activation(out_a, psA[:], TANH)
        nc.scalar.activation(out_b, psB[:], TANH)

    for t in range(nsteps + LAG):
        if t < nsteps:
            # ---- S1 step t ----
            if t % SBLK == 0:
                sblk = min(SBLK, nsteps - t)
                zxt = zxp.tile([128, sblk, KC, BL], F16, tag="f_zxt")
                nc.sync.dma_start(out=zxt[:],
                                  in_=zx_dram.ap()[:, t:t + sblk, :, :])
                state["zxt"] = zxt
            zxt = state["zxt"]
            ti = t % SBLK
            s = t % WIN
            rnn_step("s1", t, w0h_sb,
                     zxt[:, ti, 0:HC, :], zxt[:, ti, HC:KC, :],
                     hwin[:, s, 0:HC, :], hwin[:, s, HC:KC, :])
            state["s1"] = (hwin[:, s, 0:HC, :], hwin[:, s, HC:KC, :])
            # ---- inline P1 block once its h0 inputs are complete ----
            if t % TB == TB - 1:
                b = t // TB
                s0 = TB * (b % 3)
                for m in range(MC):
                    ps = p1psp.tile([128, TB, BL], F32, tag="p1_ps")
                    for k in range(KC):
                        c0 = (k * MC + m) * 128
                        nc.tensor.matmul(ps[:], w1i_sb[:, c0:c0 + 128],
                                         hwin[:, s0:s0 + TB, k, 0:BL],
                                         start=(k == 0), stop=(k == KC - 1))
                    nc.scalar.add(zwin[:, s0:s0 + TB, m, :], ps[:],
                                  zb1_sb[:, m:m + 1])
        u = t - LAG
        if 0 <= u < nsteps:
            # ---- S2 step u ----
            su = u % WIN
            if u == nsteps - 1:
                finA = hp.tile([128, HC, BW], F32, tag="finA")
                finB = hp.tile([128, HC, BW], F32, tag="finB")
                rnn_step("s2", u, w1h_sb,
                         zwin[:, su, 0:HC, :], zwin[:, su, HC:KC, :],
                         finA[:], finB[:])
                nc.sync.dma_start(out=final_param.ap()[:, 0:HC, :],
                                  in_=finA[:, :, 0:BL])
                nc.sync.dma_start(out=final_param.ap()[:, HC:KC, :],
                                  in_=finB[:, :, 0:BL])
            else:
                hna = hp.tile([128, HC, BW], F16, tag="s2_hA")
                hnb = hp.tile([128, HC, BW], F16, tag="s2_hB")
                rnn_step("s2", u, w1h_sb,
                         zwin[:, su, 0:HC, :], zwin[:, su, HC:KC, :],
                         hna[:], hnb[:])
                state["s2"] = (hna, hnb)


def _sphase(tc, nc, ctx, name, zx_dram, w_sb, ident_sb, hinit, nsteps,
            save_dram=None, final_param=None):
    """h[t] = tanh(zx[t] + h[t-1] @ W_hh), transposed layout, fully unrolled."""
    zxp = ctx.enter_context(tc.tile_pool(name=f"{name}_zx", bufs=3))
    hp = ctx.enter_context(tc.tile_pool(name=f"{name}_h", bufs=3))
    psp = ctx.enter_context(tc.tile_pool(name=f"{name}_ps", bufs=2, space="PSUM"))
    HC = KC // 2  # chunks per half
    ha, hb = hinit[:, 0:HC, :], hinit[:, HC:KC, :]  # halves of h(t-1)
    sblk = min(SBLK, nsteps)
    for t0 in range(0, nsteps, sblk):
        zxt = zxp.tile([128, sblk, KC, BL], F16)
        nc.sync.dma_start(out=zxt[:], in_=zx_dram.ap()[:, t0:t0 + sblk, :, :])
        for ti in range(sblk):
            t = t0 + ti
            # Two PSUM banks so the first tanh (half A) can run while the
            # tensor engine is still accumulating into half B.
            psA = psp.tile([128, HC, BL], F32, tag=f"{name}_psA")
            psB = psp.tile([128, HC, BL], F32, tag=f"{name}_psB")
            # Preload zx (sets has_written; independent of h(t-1), so these
            # issue during the previous step's tanh latency).
            nc.tensor.matmul(psA[:], ident_sb[:], zxt[:, ti, 0:HC, :],
                             start=True, stop=False)
            nc.tensor.matmul(psB[:], ident_sb[:], zxt[:, ti, HC:KC, :],
                             start=True, stop=False)
            # k-outer: the first 32 matmuls consume only half A of h(t-1),
            # so the sem-latency + tanh of half B hides under them.
            for k in range(KC):
                rhs = ha[:, k, :] if k < HC else hb[:, k - HC, :]
                for m in range(MC):
                    tgt = psA[:, m, :] if m < HC else psB[:, m - HC, :]
                    c0 = (k * MC + m) * 128
                    nc.tensor.matmul(tgt, w_sb[:, c0:c0 + 128], rhs,
                                     start=False, stop=(k == KC - 1))
            if t == nsteps - 1 and final_param is not None:
                finA = hp.tile([128, HC, BL], F32, tag=f"{name}_finA")
                finB = hp.tile([128, HC, BL], F32, tag=f"{name}_finB")
                nc.scalar.activation(finA[:], psA[:], TANH)
                nc.scalar.activation(finB[:], psB[:], TANH)
                nc.sync.dma_start(out=final_param.ap()[:, 0:HC, :], in_=finA[:])
                nc.sync.dma_start(out=final_param.ap()[:, HC:KC, :], in_=finB[:])
            else:
                # Separate tiles per half so consumers of half A never wait
                # on half B's tanh.
                hna = hp.tile([128, HC, BL], F16, tag=f"{name}_hA")
                hnb = hp.tile([128, HC, BL], F16, tag=f"{name}_hB")
                nc.scalar.activation(hna[:], psA[:], TANH)
                nc.scalar.activation(hnb[:], psB[:], TANH)
                if save_dram is not None:
                    nc.sync.dma_start(out=save_dram.ap()[:, t, 0:HC, :],
                                      in_=hna[:])
                    nc.sync.dma_start(out=save_dram.ap()[:, t, HC:KC, :],
                                      in_=hnb[:])
                ha, hb = hna, hnb


def _build(nsteps):
    from contextlib import ExitStack

    nc = bacc.Bacc("TRN2", target_bir_lowering=False, debug=False,
                   num_devices=NCORES)
    p = nc.declare_dram_parameter
    embT = p("embT", [128, EC, nsteps, BL], F16, False)
    w0i = p("w0i", [128, EC * MC * 128], F16, False)
    w0h = p("w0h", [128, KC * MC * 128], F16, False)
    w1i = p("w1i", [128, KC * MC * 128], F16, False)
    w1h = p("w1h", [128, KC * MC * 128], F16, False)
    zb0 = p("zb0", [128, MC], F32, False)
    zb1 = p("zb1", [128, MC], F32, False)
    ident = p("ident", [128, 128], F16, False)
    hT_out = p("hT_out", [128, KC, BL], F32, True)

    zx0 = nc.dram_tensor("zx0", [128, nsteps, KC, BL], F16)

    with tile.TileContext(nc) as tc, ExitStack() as top:
        wres = top.enter_context(tc.tile_pool(name="wres", bufs=1))
        w0h_sb = wres.tile_from(w0h.ap())
        w1h_sb = wres.tile_from(w1h.ap())
        w1i_sb = wres.tile_from(w1i.ap())
        ident_sb = wres.tile_from(ident.ap())
        zb0_sb = wres.tile_from(zb0.ap())
        zb1_sb = wres.tile_from(zb1.ap())
        hinit = wres.tile([128, KC, BW], F16)
        nc.gpsimd.memset(hinit[:], 0.0)

        with ExitStack() as ctx:
            w0ip = ctx.enter_context(tc.tile_pool(name="w0i", bufs=1))
            w0i_sb = w0ip.tile_from(w0i.ap())
            _pphase(tc, nc, ctx, "p0", w0i_sb, EC, embT, True, zx0, zb0_sb,
                    nsteps)
        tc.strict_bb_all_engine_barrier()
        with ExitStack() as ctx:
            _fused(tc, nc, ctx, zx0, w0h_sb, w1h_sb, w1i_sb, zb1_sb,
                   ident_sb, hinit, nsteps, hT_out)
    nc.compile()
    return nc


def _get_program(nsteps):
    if nsteps not in _programs:
        _programs[nsteps] = _build(nsteps)
    return _programs[nsteps]


def _wchunks(w):
    """[K, H] -> [128, K/128 * 8 * 128] with chunk (k, m) at cols (k*8+m)*128."""
    kcw = w.shape[0] // 128
    return np.ascontiguousarray(
        w.reshape(kcw, 128, MC, 128).transpose(1, 0, 2, 3).reshape(128, -1)
    ).astype(np.float16)


def _bias_cols(b):
    """[H] -> [128, MC] with b[128m+p] at [p, m]."""
    return np.ascontiguousarray(b.reshape(MC, 128).T).astype(np.float32)


def _run(inputs, nsteps):
    global last_results
    inp = {k: np.asarray(v) for k, v in inputs.items()}
    emb_x = inp["emb"].astype(np.float32)[inp["x"]]  # [B, S, E]
    emb_x = emb_x[:, :nsteps]
    ident = np.eye(128, dtype=np.float16)

    in_maps = []
    for c in range(NCORES):
        d = "fw" if c < 4 else "bw"
        b0 = BL * (c % 4)
        seq = emb_x[b0:b0 + BL]                      # [BL, nsteps, E]
        if d == "bw":
            seq = seq[:, ::-1]
        # embT[p, k, t, b] = seq[b, t, 128k+p]
        embT = np.ascontiguousarray(
            seq.transpose(2, 1, 0)                   # [E, t, b]
            .reshape(EC, 128, nsteps, BL)
            .transpose(1, 0, 2, 3)
        ).astype(np.float16)
        in_maps.append({
            "embT": embT,
            "w0i": _wchunks(inp[f"{d}0_wih"]),
            "w0h": _wchunks(inp[f"{d}0_whh"]),
            "w1i": _wchunks(inp[f"{d}1_wih"]),
            "w1h": _wchunks(inp[f"{d}1_whh"]),
            "zb0": _bias_cols(inp[f"{d}0_bih"] + inp[f"{d}0_bhh"]),
            "zb1": _bias_cols(inp[f"{d}1_bih"] + inp[f"{d}1_bhh"]),
            "ident": ident,
        })

    nc = _get_program(nsteps)
    res = run_bass_kernel_spmd(
        nc, in_maps, list(range(NCORES)),
        trace=bool(os.environ.get("BASS_TRACE")),
    )
    last_results = res

    hidden = np.zeros((B, 2 * H), dtype=np.float32)
    for c in range(NCORES):
        out = np.asarray(res.results[c]["hT_out"])   # [128, KC, BL]
        h = out.transpose(1, 0, 2).reshape(H, BL)    # [H, BL]
        b0 = BL * (c % 4)
        if c < 4:
            hidden[b0:b0 + BL, :H] = h.T
        else:
            hidden[b0:b0 + BL, H:] = h.T
    out = (hidden @ inp["fc1_w"].astype(np.float32) + inp["fc1_b"]) \
        @ inp["fc2_w"].astype(np.float32) + inp["fc2_b"]
    return out.astype(np.float32)


def kernel(**inputs):
    return _run(inputs, S)


# revision 13
# speedup vs baseline: 1.9970x; 1.0640x over previous
"""Trainium2 Bass kernel for nn_BiRNN (2-layer bidirectional tanh RNN classifier).

Strategy
--------
The output depends only on the final hidden state of the top layer in each
direction, but the tanh recurrence is strictly sequential in time.  We
restructure the per-direction compute as:

  P0: zx0[t] = emb_x[t] @ W0_ih + (b0_ih + b0_hh)      -- parallel over t
  S1: h0[t]  = tanh(zx0[t] + h0[t-1] @ W0_hh)          -- serial, 512 steps
  P1: zh1[t] = h0[t] @ W1_ih + (b1_ih + b1_hh)         -- parallel over t
  S2: h1[t]  = tanh(zh1[t] + h1[t-1] @ W1_hh)          -- serial, 512 steps

Everything is kept in *transposed* layout (hT: [H, B] with H on partitions):
each serial step streams the 64 128x128 W_hh chunks through the stationary
(fast-weight-load) port with hT as the moving operand, producing the next hT
directly -- no per-step transposes, and biases become per-partition scalars
folded into the precomputed zx terms.

All four stages run in ONE fused instruction stream per core, interleaved at
8-step granularity (P0 block -> S1 steps -> P1 block -> S2 steps, with S2
lagging S1 by 9 steps).  S1 and S2 are independent dependency chains, so each
one's tanh/semaphore latency hides under the other's matmuls, keeping the
tensor engine ~96% busy.  zx0/h0/zh1 histories live in small SBUF rings --
nothing round-trips through HBM.

The moving operand is widened to 32 columns (16 real batch + 16 don't-care
lanes) purely to keep the PE activity monitor from clock-throttling; the
don't-care lanes are initialized by matmul overwrite semantics (has_written)
and never read.

Parallelization: collectives on this hardware have multi-microsecond floors,
far too slow for 1024 per-step exchanges, so cores run independent shards:
  cores 0-3: forward direction,  batch rows 16c .. 16c+15
  cores 4-7: backward direction, batch rows 16(c-4) .. 16(c-4)+15
The tiny FC head (8.4 MFLOP) is applied on the host during unsharding.

Numerics: fp16 operands with fp32 PSUM accumulation and fp32 zx terms measure
~1e-4 relative error on the final [64, 2] output vs the fp32 reference.
"""

import os
import sys

import numpy as np

for _p in ("/opt/trn_rl_repo",):
    if _p not in sys.path:
        sys.path.insert(0, _p)

import concourse.bass as bass
import concourse.mybir as mybir
import concourse.tile as tile
from concourse import bacc
from concourse.bass_utils import run_bass_kernel_spmd

# Problem constants (hardcoded per the spec).
B, S, V, E, H, C = 64, 512, 32000, 512, 1024, 2
NCORES = 8
BL = B // 4          # batch rows per core (4-way batch split per direction)
EC = E // 128        # 4   K-chunks for the E-contraction
KC = H // 128        # 8   K-chunks for the H-contraction
MC = H // 128        # 8   output (H) chunks
BW = 32              # recurrence moving width: BL real + don't-care lanes
F16 = mybir.dt.float16
F32 = mybir.dt.float32
TANH = mybir.ActivationFunctionType.Tanh

_programs: dict = {}   # nsteps -> Bass program
last_results = None    # BassKernelResults of the most recent run (for test.py)


def _fused(tc, nc, ctx, embT, w0i_sb, w0h_sb, w1i_sb, w1h_sb, zb0_sb, zb1_sb,
           hinit, nsteps, final_param):
    TB = min(8, nsteps)             # P0/P1 block size (steps)
    WIN = 3 * TB                    # ring slots (3 blocks)
    LAG = TB + 1                    # S2 trails S1 by this many steps
    HC = KC // 2

    movp = ctx.enter_context(tc.tile_pool(name="f_mov", bufs=3))
    ringp = ctx.enter_context(tc.tile_pool(name="f_ring", bufs=1))
    hp = ctx.enter_context(tc.tile_pool(name="f_h", bufs=3))
    psp = ctx.enter_context(tc.tile_pool(name="f_ps", bufs=2, space="PSUM"))
    psp2 = ctx.enter_context(tc.tile_pool(name="f_ps2", bufs=1, space="PSUM"))
    ppsp = ctx.enter_context(tc.tile_pool(name="f_pps", bufs=2, space="PSUM"))

    xwin = ringp.tile([128, WIN, KC, BL], F32)   # zx0 ring
    hwin = ringp.tile([128, WIN, KC, BW], F16)   # h0 history ring (wide)
    zwin = ringp.tile([128, WIN, MC, BL], F32)   # zh1 ring

    state = {
        "s1": (hinit[:, 0:HC, :], hinit[:, HC:KC, :]),
        "s2": (hinit[:, 0:HC, :], hinit[:, HC:KC, :]),
    }

    def rnn_step(which, w_sb_, zx_a, zx_b, out_a, out_b):
        """psum = W_hh^T h(t-1); psum += zx; out = tanh(psum)."""
        ha, hb = state[which]
        pool = psp if which == "s1" else psp2
        psA = pool.tile([128, HC, BW], F32, tag=f"{which}_psA")
        psB = pool.tile([128, HC, BW], F32, tag=f"{which}_psB")
        for k in range(KC):
            rhs = ha[:, k, :] if k < HC else hb[:, k - HC, :]
            for m in range(MC):
                tgt = psA[:, m, :] if m < HC else psB[:, m - HC, :]
                c0 = (k * MC + m) * 128
                # start=True on each bank's first matmul clears has_written
                # for the whole bank; the remaining k=0 matmuls then
                # overwrite (bit unset) and k>0 accumulate.
                nc.tensor.matmul(tgt, w_sb_[:, c0:c0 + 128], rhs,
                                 start=(k == 0 and m % HC == 0),
                                 stop=(k == KC - 1))
        nc.vector.tensor_add(psA[:, :, 0:BL], psA[:, :, 0:BL], zx_a)
        nc.vector.tensor_add(psB[:, :, 0:BL], psB[:, :, 0:BL], zx_b)
        nc.scalar.activation(out_a, psA[:], TANH)
        nc.scalar.activation(out_b, psB[:], TANH)

    def p0_block(b):
        s0 = TB * (b % 3)
        t0 = b * TB
        mov = movp.tile([128, EC, TB, BL], F16, tag="p0_mov")
        nc.sync.dma_start(out=mov[:], in_=embT.ap()[:, :, t0:t0 + TB, :])
        for m in range(MC):
            ps = ppsp.tile([128, TB, BL], F32, tag="pp_ps")
            for k in range(EC):
                c0 = (k * MC + m) * 128
                nc.tensor.matmul(ps[:], w0i_sb[:, c0:c0 + 128],
                                 mov[:, k, :, :],
                                 start=(k == 0), stop=(k == EC - 1))
            nc.scalar.add(xwin[:, s0:s0 + TB, m, :], ps[:], zb0_sb[:, m:m + 1])

    def p1_block(b):
        s0 = TB * (b % 3)
        for m in range(MC):
            ps = ppsp.tile([128, TB, BL], F32, tag="pp_ps")
            for k in range(KC):
                c0 = (k * MC + m) * 128
                nc.tensor.matmul(ps[:], w1i_sb[:, c0:c0 + 128],
                                 hwin[:, s0:s0 + TB, k, 0:BL],
                                 start=(k == 0), stop=(k == KC - 1))
            nc.scalar.add(zwin[:, s0:s0 + TB, m, :], ps[:], zb1_sb[:, m:m + 1])

    for t in range(nsteps + LAG):
        if t < nsteps:
            if t % TB == 0:
                p0_block(t // TB)           # feeds S1 steps t .. t+TB-1
            s = t % WIN
            rnn_step("s1", w0h_sb,
                     xwin[:, s, 0:HC, :], xwin[:, s, HC:KC, :],
                     hwin[:, s, 0:HC, :], hwin[:, s, HC:KC, :])
            state["s1"] = (hwin[:, s, 0:HC, :], hwin[:, s, HC:KC, :])
            if t % TB == TB - 1:
                p1_block(t // TB)           # consumes S1 steps t-TB+1 .. t
        u = t - LAG
        if 0 <= u < nsteps:
            su = u % WIN
            if u == nsteps - 1:
                finA = hp.tile([128, HC, BW], F32, tag="finA")
                finB = hp.tile([128, HC, BW], F32, tag="finB")
                rnn_step("s2", w1h_sb,
                         zwin[:, su, 0:HC, :], zwin[:, su, HC:KC, :],
                         finA[:], finB[:])
                nc.sync.dma_start(out=final_param.ap()[:, 0:HC, :],
                                  in_=finA[:, :, 0:BL])
                nc.sync.dma_start(out=final_param.ap()[:, HC:KC, :],
                                  in_=finB[:, :, 0:BL])
            else:
                hna = hp.tile([128, HC, BW], F16, tag="s2_hA")
                hnb = hp.tile([128, HC, BW], F16, tag="s2_hB")
                rnn_step("s2", w1h_sb,
                         zwin[:, su, 0:HC, :], zwin[:, su, HC:KC, :],
                         hna[:], hnb[:])
                state["s2"] = (hna, hnb)


def _build(nsteps):
    from contextlib import ExitStack

    nc = bacc.Bacc("TRN2", target_bir_lowering=False, debug=False,
                   num_devices=NCORES)
    p = nc.declare_dram_parameter
    embT = p("embT", [128, EC, nsteps, BL], F16, False)
    w0i = p("w0i", [128, EC * MC * 128], F16, False)
    w0h = p("w0h", [128, KC * MC * 128], F16, False)
    w1i = p("w1i", [128, KC * MC * 128], F16, False)
    w1h = p("w1h", [128, KC * MC * 128], F16, False)
    zb0 = p("zb0", [128, MC], F32, False)
    zb1 = p("zb1", [128, MC], F32, False)
    hT_out = p("hT_out", [128, KC, BL], F32, True)

    with tile.TileContext(nc) as tc, ExitStack() as top:
        wres = top.enter_context(tc.tile_pool(name="wres", bufs=1))
        # First-needed tiles first so their DMAs aren't queued behind the
        # big weight loads.
        w0i_sb = wres.tile_from(w0i.ap())
        zb0_sb = wres.tile_from(zb0.ap())
        w0h_sb = wres.tile_from(w0h.ap())
        zb1_sb = wres.tile_from(zb1.ap())
        w1i_sb = wres.tile_from(w1i.ap())
        w1h_sb = wres.tile_from(w1h.ap())
        hinit = wres.tile([128, KC, BW], F16)
        nc.gpsimd.memset(hinit[:], 0.0)

        with ExitStack() as ctx:
            _fused(tc, nc, ctx, embT, w0i_sb, w0h_sb, w1i_sb, w1h_sb,
                   zb0_sb, zb1_sb, hinit, nsteps, hT_out)
    nc.compile()
    return nc


def _get_program(nsteps):
    if nsteps not in _programs:
        _programs[nsteps] = _build(nsteps)
    return _programs[nsteps]


def _wchunks(w):
    """[K, H] -> [128, K/128 * 8 * 128] with chunk (k, m) at cols (k*8+m)*128."""
    kcw = w.shape[0] // 128
    return np.ascontiguousarray(
        w.reshape(kcw, 128, MC, 128).transpose(1, 0, 2, 3).reshape(128, -1)
    ).astype(np.float16)


def _bias_cols(b):
    """[H] -> [128, MC] with b[128m+p] at [p, m]."""
    return np.ascontiguousarray(b.reshape(MC, 128).T).astype(np.float32)


def _run(inputs, nsteps):
    global last_results
    inp = {k: np.asarray(v) for k, v in inputs.items()}
    emb_x = inp["emb"].astype(np.float32)[inp["x"]]  # [B, S, E]
    emb_x = emb_x[:, :nsteps]

    in_maps = []
    for c in range(NCORES):
        d = "fw" if c < 4 else "bw"
        b0 = BL * (c % 4)
        seq = emb_x[b0:b0 + BL]                      # [BL, nsteps, E]
        if d == "bw":
            seq = seq[:, ::-1]
        # embT[p, k, t, b] = seq[b, t, 128k+p]
        embT = np.ascontiguousarray(
            seq.transpose(2, 1, 0)                   # [E, t, b]
            .reshape(EC, 128, nsteps, BL)
            .transpose(1, 0, 2, 3)
        ).astype(np.float16)
        in_maps.append({
            "embT": embT,
            "w0i": _wchunks(inp[f"{d}0_wih"]),
            "w0h": _wchunks(inp[f"{d}0_whh"]),
            "w1i": _wchunks(inp[f"{d}1_wih"]),
            "w1h": _wchunks(inp[f"{d}1_whh"]),
            "zb0": _bias_cols(inp[f"{d}0_bih"] + inp[f"{d}0_bhh"]),
            "zb1": _bias_cols(inp[f"{d}1_bih"] + inp[f"{d}1_bhh"]),
        })

    nc = _get_program(nsteps)
    res = run_bass_kernel_spmd(
        nc, in_maps, list(range(NCORES)),
        trace=bool(os.environ.get("BASS_TRACE")),
    )
    last_results = res

    hidden = np.zeros((B, 2 * H), dtype=np.float32)
    for c in range(NCORES):
        out = np.asarray(res.results[c]["hT_out"])   # [128, KC, BL]
        h = out.transpose(1, 0, 2).reshape(H, BL)    # [H, BL]
        b0 = BL * (c % 4)
        if c < 4:
            hidden[b0:b0 + BL, :H] = h.T
        else:
            hidden[b0:b0 + BL, H:] = h.T
    out = (hidden @ inp["fc1_w"].astype(np.float32) + inp["fc1_b"]) \
        @ inp["fc2_w"].astype(np.float32) + inp["fc2_b"]
    return out.astype(np.float32)


def kernel(**inputs):
    return _run(inputs, S)
